# revision 21
# baseline (speedup 1.0000x reference)
"""AdaIN (CodeFormer) Trainium2 Bass kernel.

out[b,c,:,:] = (soft[b,c] - mean(soft[b,c])) / std(soft[b,c]) * std(z[b,c]) + mean(z[b,c])

std is unbiased (ddof=1), clamped to EPS=1e-5. Stats over the flattened 64*64
spatial dim, fp32 throughout.

Sharding: pure data parallelism over the batch dim. B=16 across 8 cores ->
2 batches/core = 1024 (b,c) rows of 4096 elements each, processed as 8 tiles
of [128 partitions x 4096].

The kernel is memory-bound: 50.3 MiB of HBM traffic/core against a measured
~430 GB/s aggregate DMA ceiling (~117 us floor). Scheduling notes:
  - ALL DMA rides the sync engine's HWDGE ring (qSPDynamicHW) as flat
    per-tile 2 MiB transfers. The sync sequencer has no compute, so blocking
    at the shallow trigger FIFO is harmless, and ring FIFO order defers the
    stores behind the loads with zero semaphore gap - loads get exclusive
    bandwidth, then the stores stream back-to-back.
  - SWDGE (gpsimd) is avoided entirely: its SBUF descriptor rings contend
    with SDMA engine 15's AXI port and make it straggle ~20us behind the
    other queues. Multi-dim store APs (e.g. [p, k, f] pair stores) hit the
    same engine-15 pathology even on HWDGE - keep transfers flat.
  - ACT's cross-engine waits are bound to FRONT-block DVE products
    (aggr/zs2n, ready at data-arrival pace); the finish chain is pure DVE
    one tile behind, so the in-order DVE/ACT streams never bubble.
  - The remaining tail is the TileContext epilogue: each sequencer serially
    retires ~55 semaphore events (~0.1 us each) gated on the final store
    completions (HBM write-receipt ~1-2 us). Attempts to shrink the event
    count via batched DMAs / per-pair scalar chains cost more in pipeline
    quantization than they save here.
"""

import numpy as np

import bass_rust
import concourse.bass as bass
import concourse.tile as tile
from concourse import mybir
from concourse.bass_utils import run_bass_kernel_spmd

B, C, H, W = 16, 512, 64, 64
EPS = 1e-5
N_CORES = 8
SPATIAL = H * W  # 4096
ROWS = (B // N_CORES) * C  # 1024 rows per core
P = 128
N_TILES = ROWS // P  # 8
BN_SEG = 512  # bn_stats hardware free-dim limit
N_SEG = SPATIAL // BN_SEG  # 8
DDOF_CORR = float(SPATIAL) / float(SPATIAL - 1)  # unbiased variance factor

F32 = mybir.dt.float32


def _split_multiwait_insts(nc: bass.Bass) -> int:
    """The stock walrus in this container allows only one sync-wait slot per
    instruction ("Too many sync wait commands" otherwise). Tile emits
    multi-wait sync_info; hoist all but the last wait onto standalone NoOps
    on the same engine, immediately before the owning instruction."""
    m = nc.m
    total = 0
    for fi, f in enumerate(m.functions):
        blocks = f.blocks
        changed = False
        for blk in blocks:
            insts = blk.instructions
            new_insts = []
            blk_changed = False
            for ins in insts:
                si = ins.sync_info
                waits = list(si.on_wait) if si is not None and si.on_wait else []
                if len(waits) > 1:
                    for w in waits[:-1]:
                        total += 1
                        new_insts.append(
                            bass_rust.InstNoOp(
                                name=f"I-mwsplit-{total}",
                                engine=ins.engine,
                                sync_info=bass_rust.SyncInfo(
                                    on_wait=[w], on_update=[]
                                ),
                            )
                        )
                    ins.sync_info = bass_rust.SyncInfo(
                        on_wait=[waits[-1]],
                        on_update=list(si.on_update) if si.on_update else [],
                    )
                    blk_changed = True
                new_insts.append(ins)
            if blk_changed:
                blk.instructions = new_insts
                changed = True
        if changed:
            f.blocks = blocks
            m.functions[fi] = f
    return total


def _build_nc() -> bass.Bass:
    nc = bass.Bass()
    soft = nc.dram_tensor("soft", [ROWS, SPATIAL], F32, kind="ExternalInput")
    z = nc.dram_tensor("z", [ROWS, SPATIAL], F32, kind="ExternalInput")
    out = nc.dram_tensor("out", [ROWS, SPATIAL], F32, kind="ExternalOutput")

    # Constants for the z-stats path (sum/sumsq accumulated on ScalarE):
    #   z_var_unbiased = z_sumsq/(n-1) - z_sum^2/(n*(n-1))
    n = float(SPATIAL)
    c1 = 1.0 / (n - 1.0)
    c2 = 1.0 / (n * (n - 1.0))
    c3 = 1.0 / n

    with tile.TileContext(nc) as tc:
        with (
            tc.tile_pool(name="softp", bufs=N_TILES) as softp,
            tc.tile_pool(name="zp", bufs=4) as zp,
            tc.tile_pool(name="stats", bufs=4) as stats,
        ):
            # All 8 soft loads first on the sync HWDGE ring (the sync
            # sequencer has no compute, so blocking at the shallow trigger
            # FIFO is harmless).
            soft_tiles = []
            for it in range(N_TILES):
                rows = slice(it * P, (it + 1) * P)
                soft_t = softp.tile([P, SPATIAL], F32, tag="soft")
                nc.sync.dma_start(out=soft_t, in_=soft[rows, :])
                soft_tiles.append(soft_t)

            # z loads on the scalar (ACT) HWDGE ring so both rings generate
            # descriptors concurrently. Each trigger is emitted THREE tiles
            # ahead of the Copy/Square chain that consumes it, so ACT always
            # has ~2 transfers queued but never sits blocked on FIFO-accept
            # for long; the zp slot-recycle wait (Square of 4 tiles earlier)
            # is pre-satisfied one chain before the trigger.
            z_tiles = [None] * N_TILES

            def trigger_z(it):
                rows = slice(it * P, (it + 1) * P)
                z_t = zp.tile([P, SPATIAL], F32, tag="z")
                nc.scalar.dma_start(out=z_t, in_=z[rows, :])
                z_tiles[it] = z_t

            def front(it):
                """Heavy one-pass stats + cross-engine sqrt chain for tile
                `it` (loads already in flight). Returns state the finishing
                stage needs."""
                soft_t = soft_tiles[it]
                z_t = z_tiles[it]

                # soft stats: per-row mean/var via bn_stats (VectorE), one pass.
                s_stats = stats.tile([P, N_SEG, 6], F32, tag="s_stats")
                soft_seg = soft_t[:, :].rearrange("p (g f) -> p g f", f=BN_SEG)
                for g in range(N_SEG):
                    nc.vector.bn_stats(out=s_stats[:, g, :], in_=soft_seg[:, g, :])
                s_mv = stats.tile([P, 2], F32, tag="s_mv")
                nc.vector.bn_aggr(out=s_mv, in_=s_stats)

                # z stats on ScalarE: sum via in-place Copy, then sumsq via
                # in-place Square (z is dead after this).
                z_sum = stats.tile([P, 1], F32, tag="z_sum")
                z_sumsq = stats.tile([P, 1], F32, tag="z_sumsq")
                nc.scalar.activation(
                    out=z_t, in_=z_t,
                    func=mybir.ActivationFunctionType.Copy, accum_out=z_sum,
                )
                nc.scalar.activation(
                    out=z_t, in_=z_t,
                    func=mybir.ActivationFunctionType.Square, accum_out=z_sumsq,
                )

                # s_std = sqrt(s_var * n/(n-1)), z_std = sqrt(z_sumsq*c1 - z_sum^2*c2)
                s_std = stats.tile([P, 1], F32, tag="s_std")
                zs2n = stats.tile([P, 1], F32, tag="zs2n")
                z_std = stats.tile([P, 1], F32, tag="z_std")
                nc.scalar.activation(
                    out=s_std, in_=s_mv[:, 1:2],
                    func=mybir.ActivationFunctionType.Sqrt, scale=DDOF_CORR,
                )
                nc.vector.tensor_mul(out=zs2n, in0=z_sum, in1=z_sum)
                nc.vector.tensor_scalar_mul(out=zs2n, in0=zs2n, scalar1=-c2)
                nc.scalar.activation(
                    out=z_std, in_=z_sumsq,
                    func=mybir.ActivationFunctionType.Sqrt, scale=c1, bias=zs2n,
                )
                return it, soft_t, s_mv, z_sum, s_std, z_std

            def finish(state):
                """EPS clamps, a/b scalars, fused normalize — pure DVE,
                emitted one tile behind `front` so every cross-engine wait is
                pre-satisfied and the in-order DVE/ACT streams never bubble."""
                it, soft_t, s_mv, z_sum, s_std, z_std = state

                nc.vector.tensor_scalar_max(out=s_std, in0=s_std, scalar1=EPS)
                nc.vector.tensor_scalar_max(out=z_std, in0=z_std, scalar1=EPS)

                # a = z_std / s_std ;  b = z_sum*c3 - s_mean * a
                rcp = stats.tile([P, 1], F32, tag="rcp")
                a_sc = stats.tile([P, 1], F32, tag="a_sc")
                b_sc = stats.tile([P, 1], F32, tag="b_sc")
                nc.vector.reciprocal(out=rcp, in_=s_std)
                nc.vector.tensor_mul(out=a_sc, in0=z_std, in1=rcp)
                nc.vector.tensor_mul(out=b_sc, in0=s_mv[:, 0:1], in1=a_sc)
                nc.vector.scalar_tensor_tensor(
                    out=b_sc, in0=z_sum, scalar=c3, in1=b_sc,
                    op0=mybir.AluOpType.mult, op1=mybir.AluOpType.subtract,
                )

                # out = soft * a + b  (single fused pass, in place)
                nc.vector.tensor_scalar(
                    out=soft_t, in0=soft_t,
                    scalar1=a_sc, scalar2=b_sc,
                    op0=mybir.AluOpType.mult, op1=mybir.AluOpType.add,
                )
                return soft_t

            done = []
            pending = None
            for it in range(3):
                trigger_z(it)
            for it in range(N_TILES):
                state = front(it)
                if it + 3 < N_TILES:
                    trigger_z(it + 3)
                if pending is not None:
                    done.append(finish(pending))
                pending = state
            done.append(finish(pending))

            # Stores on the sync ring, emitted after all loads in the sync
            # engine stream: ring FIFO order defers them behind the loads with
            # zero gap. Each store's wait (normalize of its tile, DVE) is
            # pre-satisfied because the staggered finish() blocks complete at
            # load pace.
            for it, soft_t in enumerate(done):
                rows = slice(it * P, (it + 1) * P)
                nc.sync.dma_start(out=out[rows, :], in_=soft_t)

    _split_multiwait_insts(nc)
    return nc


def _run(soft: np.ndarray, z: np.ndarray, trace: bool = False):
    nc = _build_nc()
    soft_flat = np.ascontiguousarray(np.asarray(soft, dtype=np.float32)).reshape(
        B * C, SPATIAL
    )
    z_flat = np.ascontiguousarray(np.asarray(z, dtype=np.float32)).reshape(
        B * C, SPATIAL
    )
    in_maps = [
        {
            "soft": soft_flat[k * ROWS : (k + 1) * ROWS],
            "z": z_flat[k * ROWS : (k + 1) * ROWS],
        }
        for k in range(N_CORES)
    ]
    res = run_bass_kernel_spmd(nc, in_maps, core_ids=list(range(N_CORES)), trace=trace)
    out = np.concatenate([r["out"] for r in res.results], axis=0)
    return out.reshape(B, C, H, W), res


def kernel(soft: np.ndarray, z: np.ndarray) -> np.ndarray:
    out, _ = _run(soft, z, trace=False)
    return out
